# revision 5
# baseline (speedup 1.0000x reference)
"""Trainium2 Bass kernel for a 2-layer GRU autoregressive decoder.

Problem shape: latent [512, 128, 256] -> fused batch N = 65536 independent
samples, K = 50 sequential decode steps each. Per step, per sample:
  x   = concat(lat, prev)                       # prev = previous state output
  h0  = GRUCell(x,  h0)   (hidden 128)
  h1  = GRUCell(h0, h1)   (hidden 128)
  out = W_out @ h1 + b_out   (6)  ;  stop = sigmoid(W_stop @ h1 + b_stop)

Device strategy (data parallel over 8 cores, 8192 samples each):
  * Layout: feature dim on SBUF partitions, samples on the free dim.
  * base = W_ih0[:, :256] @ lat^T + b_ih0 precomputed once per sample
    (latent is constant across the 50 steps).
  * prev only enters through W_ih0[:, 256:262] @ (W_out @ h1 + b_out), so
    W_fuse = W_ih0[:, 256:262] @ W_out turns the feedback into a plain
    128-contraction matmul on h1 (head taken off the critical path).
    Constant terms (b_fuse, and step-0's W_ih0p @ start_token) become ACT
    bias vectors.
  * Per 512-sample block and step: all gate pre-activations accumulate in
    PSUM (weight matmuls + identity-matmul adds of SBUF tensors), sigmoids
    and tanhs on ScalarE with fused per-partition biases,
    (hn + b_hh_n) * r fused into one scalar_tensor_tensor on VectorE, and
    the GRU state update h' = n + z*(h-n) done 4-blocks-wide on VectorE.
  * Head rows [out(6); stop_logit(1)] are evacuated PSUM->SBUF with the
    output bias fused, then DMAd to DRAM as [K, 7, 8192]; the final
    transpose to [B, T, K, ...] and the stop sigmoid are host-side
    reshaping of the gathered result.
"""

import sys

if "/opt/trn_rl_repo" not in sys.path:
    sys.path.insert(0, "/opt/trn_rl_repo")

import numpy as np

import concourse.bacc as bacc
import concourse.tile as tile
from concourse import mybir
from concourse import bass_utils

P = 128
FD = 512                 # samples per block (one PSUM bank of fp32)
GRP = 2                  # blocks per update group (wide DVE ops)
R = 4096                 # samples per round (SBUF residency of base/h)
ROUNDS = 2
NBLK = R // FD           # 8 blocks per round
K = 50                   # decode steps
NCORES = 8
NS = ROUNDS * R          # samples per core = 8192
LAT = 256
H = 128
F32 = mybir.dt.float32

AF = mybir.ActivationFunctionType
ALU = mybir.AluOpType

# bias column indices in the packed [128, 15] bias matrix
B_R0_K0, B_R0, B_Z0_K0, B_Z0, B_STT0, B_N0_K0, B_N0 = 0, 1, 2, 3, 4, 5, 6
B_R1, B_Z1, B_STT1, B_N1, B_HEAD = 7, 8, 9, 10, 11
B_BASE0, B_BASE1, B_BASE2 = 12, 13, 14
NBIAS = 15

_CACHED = None


def _build_program():
    nc = bacc.Bacc("TRN2", target_bir_lowering=False, debug=False,
                   num_devices=NCORES)

    latT_d = nc.dram_tensor("latT", [ROUNDS, NBLK, 2, P, FD], F32,
                            kind="ExternalInput")
    whh0_d = nc.dram_tensor("whh0T", [P, 3, H], F32, kind="ExternalInput")
    wfuse_d = nc.dram_tensor("wfuseT", [P, 3, H], F32, kind="ExternalInput")
    wih1_d = nc.dram_tensor("wih1T", [P, 3, H], F32, kind="ExternalInput")
    whh1_d = nc.dram_tensor("whh1T", [P, 3, H], F32, kind="ExternalInput")
    whead_d = nc.dram_tensor("wheadT", [P, 7], F32, kind="ExternalInput")
    wlat_d = nc.dram_tensor("wlatT", [P, 2, 3, H], F32, kind="ExternalInput")
    ident_d = nc.dram_tensor("ident", [P, P], F32, kind="ExternalInput")
    bias_d = nc.dram_tensor("biases", [P, NBIAS], F32, kind="ExternalInput")
    out_d = nc.dram_tensor("outsd", [K, 7, NS], F32, kind="ExternalOutput")
    out_ap = out_d.ap()

    with tile.TileContext(nc) as tc:
        with (
            tc.tile_pool(name="consts", bufs=1) as consts,
            tc.tile_pool(name="statep", bufs=1) as statep,
            tc.tile_pool(name="basep", bufs=1) as basep,
            tc.tile_pool(name="work", bufs=2) as work,
            tc.tile_pool(name="widep", bufs=2) as widep,
            tc.tile_pool(name="lats", bufs=2) as lats,
            tc.tile_pool(name="outsp", bufs=3) as outsp,
            tc.tile_pool(name="psum", bufs=8, space="PSUM") as psum,
        ):
            whh0 = consts.tile([P, 3, H], F32)
            nc.sync.dma_start(whh0[:], whh0_d.ap())
            wfuse = consts.tile([P, 3, H], F32)
            nc.sync.dma_start(wfuse[:], wfuse_d.ap())
            wih1 = consts.tile([P, 3, H], F32)
            nc.sync.dma_start(wih1[:], wih1_d.ap())
            whh1 = consts.tile([P, 3, H], F32)
            nc.sync.dma_start(whh1[:], whh1_d.ap())
            whead = consts.tile([P, 7], F32)
            nc.sync.dma_start(whead[:], whead_d.ap())
            wlat = consts.tile([P, 2, 3, H], F32)
            nc.sync.dma_start(wlat[:], wlat_d.ap())
            ident = consts.tile([P, P], F32)
            nc.sync.dma_start(ident[:], ident_d.ap())
            biases = consts.tile([P, NBIAS], F32)
            nc.sync.dma_start(biases[:], bias_d.ap())

            def bcol(j, parts=P):
                return biases[0:parts, j:j + 1]

            for rnd in range(ROUNDS):
                h0 = statep.tile([P, R], F32, tag="h0")
                h1 = statep.tile([P, R], F32, tag="h1")
                nc.gpsimd.memset(h0[:], 0.0)
                nc.gpsimd.memset(h1[:], 0.0)
                base = basep.tile([P, 3, R], F32, tag="base")

                # ---- precompute base = W_lat @ lat^T + b_ih0 ----
                for b in range(NBLK):
                    cs = slice(b * FD, (b + 1) * FD)
                    lat0 = lats.tile([P, FD], F32, tag="lat0")
                    nc.sync.dma_start(lat0[:], latT_d.ap()[rnd, b, 0])
                    lat1 = lats.tile([P, FD], F32, tag="lat1")
                    nc.sync.dma_start(lat1[:], latT_d.ap()[rnd, b, 1])
                    for g in range(3):
                        ps = psum.tile([P, FD], F32, tag="ps")
                        nc.tensor.matmul(ps[:], wlat[:, 0, g, :], lat0[:],
                                         start=True, stop=False)
                        nc.tensor.matmul(ps[:], wlat[:, 1, g, :], lat1[:],
                                         start=False, stop=True)
                        nc.scalar.activation(base[:, g, cs], ps[:],
                                             AF.Identity, bias=bcol(B_BASE0 + g))

                # ---- K decode steps ----
                for k in range(K):
                    for grp in range(NBLK // GRP):
                        gcs = slice(grp * GRP * FD, (grp + 1) * GRP * FD)
                        z0w = widep.tile([P, GRP * FD], F32, tag="z0w")
                        n0w = widep.tile([P, GRP * FD], F32, tag="n0w")

                        # ----- layer 0 (reads h0, h1 old) -----
                        for q in range(GRP):
                            b = grp * GRP + q
                            cs = slice(b * FD, (b + 1) * FD)
                            qs = slice(q * FD, (q + 1) * FD)
                            h0b = h0[:, cs]
                            h1b = h1[:, cs]

                            ps_r = psum.tile([P, FD], F32, tag="ps")
                            nc.tensor.matmul(ps_r[:], whh0[:, 0, :], h0b,
                                             start=True, stop=False)
                            nc.tensor.matmul(ps_r[:], wfuse[:, 0, :], h1b,
                                             start=False, stop=False)
                            nc.tensor.matmul(ps_r[:], ident[:], base[:, 0, cs],
                                             start=False, stop=True)
                            r0 = work.tile([P, FD], F32, tag="r0")
                            nc.scalar.activation(
                                r0[:], ps_r[:], AF.Sigmoid,
                                bias=bcol(B_R0_K0 if k == 0 else B_R0))

                            ps_z = psum.tile([P, FD], F32, tag="ps")
                            nc.tensor.matmul(ps_z[:], whh0[:, 1, :], h0b,
                                             start=True, stop=False)
                            nc.tensor.matmul(ps_z[:], wfuse[:, 1, :], h1b,
                                             start=False, stop=False)
                            nc.tensor.matmul(ps_z[:], ident[:], base[:, 1, cs],
                                             start=False, stop=True)
                            nc.scalar.activation(
                                z0w[:, qs], ps_z[:], AF.Sigmoid,
                                bias=bcol(B_Z0_K0 if k == 0 else B_Z0))

                            ps_hn = psum.tile([P, FD], F32, tag="ps")
                            nc.tensor.matmul(ps_hn[:], whh0[:, 2, :], h0b,
                                             start=True, stop=True)
                            tmp0 = work.tile([P, FD], F32, tag="tmp0")
                            # tmp0 = (hn_psum + b_hh0_n) * r0
                            nc.vector.scalar_tensor_tensor(
                                tmp0[:], ps_hn[:], bcol(B_STT0), r0[:],
                                op0=ALU.add, op1=ALU.mult)

                            ps_in = psum.tile([P, FD], F32, tag="ps")
                            nc.tensor.matmul(ps_in[:], wfuse[:, 2, :], h1b,
                                             start=True, stop=False)
                            nc.tensor.matmul(ps_in[:], ident[:], base[:, 2, cs],
                                             start=False, stop=False)
                            nc.tensor.matmul(ps_in[:], ident[:], tmp0[:],
                                             start=False, stop=True)
                            nc.scalar.activation(
                                n0w[:, qs], ps_in[:], AF.Tanh,
                                bias=bcol(B_N0_K0 if k == 0 else B_N0))

                        # ----- update h0 (group-wide): h0' = n + z*(h0-n) ---
                        d0 = widep.tile([P, GRP * FD], F32, tag="d0", bufs=1)
                        nc.vector.tensor_sub(d0[:], h0[:, gcs], n0w[:])
                        nc.vector.tensor_mul(d0[:], z0w[:], d0[:])
                        nc.vector.tensor_add(h0[:, gcs], n0w[:], d0[:])

                        # ----- layer 1 (reads h0 new, h1 old) -----
                        z1w = widep.tile([P, GRP * FD], F32, tag="z1w")
                        n1w = widep.tile([P, GRP * FD], F32, tag="n1w")
                        for q in range(GRP):
                            b = grp * GRP + q
                            cs = slice(b * FD, (b + 1) * FD)
                            qs = slice(q * FD, (q + 1) * FD)
                            h0b = h0[:, cs]
                            h1b = h1[:, cs]

                            ps_r = psum.tile([P, FD], F32, tag="ps")
                            nc.tensor.matmul(ps_r[:], wih1[:, 0, :], h0b,
                                             start=True, stop=False)
                            nc.tensor.matmul(ps_r[:], whh1[:, 0, :], h1b,
                                             start=False, stop=True)
                            r1 = work.tile([P, FD], F32, tag="r1")
                            nc.scalar.activation(r1[:], ps_r[:], AF.Sigmoid,
                                                 bias=bcol(B_R1))

                            ps_z = psum.tile([P, FD], F32, tag="ps")
                            nc.tensor.matmul(ps_z[:], wih1[:, 1, :], h0b,
                                             start=True, stop=False)
                            nc.tensor.matmul(ps_z[:], whh1[:, 1, :], h1b,
                                             start=False, stop=True)
                            nc.scalar.activation(z1w[:, qs], ps_z[:],
                                                 AF.Sigmoid, bias=bcol(B_Z1))

                            ps_hn = psum.tile([P, FD], F32, tag="ps")
                            nc.tensor.matmul(ps_hn[:], whh1[:, 2, :], h1b,
                                             start=True, stop=True)
                            tmp1 = work.tile([P, FD], F32, tag="tmp1")
                            nc.vector.scalar_tensor_tensor(
                                tmp1[:], ps_hn[:], bcol(B_STT1), r1[:],
                                op0=ALU.add, op1=ALU.mult)

                            ps_in = psum.tile([P, FD], F32, tag="ps")
                            nc.tensor.matmul(ps_in[:], wih1[:, 2, :], h0b,
                                             start=True, stop=False)
                            nc.tensor.matmul(ps_in[:], ident[:], tmp1[:],
                                             start=False, stop=True)
                            nc.scalar.activation(n1w[:, qs], ps_in[:],
                                                 AF.Tanh, bias=bcol(B_N1))

                        # ----- update h1 (group-wide) -----
                        d1 = widep.tile([P, GRP * FD], F32, tag="d1", bufs=1)
                        nc.vector.tensor_sub(d1[:], h1[:, gcs], n1w[:])
                        nc.vector.tensor_mul(d1[:], z1w[:], d1[:])
                        nc.vector.tensor_add(h1[:, gcs], n1w[:], d1[:])

                        # ----- head (reads h1 new) -----
                        for q in range(GRP):
                            b = grp * GRP + q
                            cs = slice(b * FD, (b + 1) * FD)
                            ps_h = psum.tile([7, FD], F32, tag="ps")
                            nc.tensor.matmul(ps_h[:], whead[:], h1[:, cs],
                                             start=True, stop=True)
                            ost = outsp.tile([7, FD], F32, tag="ost")
                            nc.scalar.activation(ost[:], ps_h[:], AF.Identity,
                                                 bias=bcol(B_HEAD, parts=7))
                            col0 = rnd * R + b * FD
                            nc.sync.dma_start(
                                out_ap[k, :, col0:col0 + FD], ost[:])

    nc.compile()
    return nc


def _prep_weights(W_ih0, W_hh0, b_ih0, b_hh0, W_ih1, W_hh1, b_ih1, b_hh1,
                  W_out, b_out, W_stop, b_stop, start_token):
    f32 = np.float32
    Wlat = np.ascontiguousarray(W_ih0[:, :LAT])
    Wp = W_ih0[:, LAT:].astype(np.float64)
    W_fuse = (Wp @ W_out.astype(np.float64)).astype(f32)
    b_fuse = (Wp @ b_out.astype(np.float64)).astype(f32)
    c0 = (Wp @ start_token[0].astype(np.float64)).astype(f32)

    def lhsT3(W):  # [384, 128] -> [128(contract), 3(gate), 128(out)]
        return np.ascontiguousarray(
            np.transpose(W.reshape(3, H, H), (2, 0, 1)).astype(f32))

    whh0T = lhsT3(W_hh0)
    wfuseT = lhsT3(W_fuse)
    wih1T = lhsT3(W_ih1)
    whh1T = lhsT3(W_hh1)
    wheadT = np.ascontiguousarray(
        np.vstack([W_out, W_stop]).T.astype(f32))          # [128, 7]
    # [128(contract within chunk), 2(chunk), 3(gate), 128(out)]
    wlatT = np.ascontiguousarray(
        np.transpose(Wlat.reshape(3, H, 2, P), (3, 2, 0, 1)).astype(f32))
    ident = np.eye(P, dtype=f32)

    def bg(v, g):
        return v[g * H:(g + 1) * H]

    cols = np.zeros((P, NBIAS), dtype=f32)
    cols[:, B_R0_K0] = bg(b_hh0, 0) + bg(c0, 0)
    cols[:, B_R0] = bg(b_hh0, 0) + bg(b_fuse, 0)
    cols[:, B_Z0_K0] = bg(b_hh0, 1) + bg(c0, 1)
    cols[:, B_Z0] = bg(b_hh0, 1) + bg(b_fuse, 1)
    cols[:, B_STT0] = bg(b_hh0, 2)
    cols[:, B_N0_K0] = bg(c0, 2)
    cols[:, B_N0] = bg(b_fuse, 2)
    cols[:, B_R1] = bg(b_ih1, 0) + bg(b_hh1, 0)
    cols[:, B_Z1] = bg(b_ih1, 1) + bg(b_hh1, 1)
    cols[:, B_STT1] = bg(b_hh1, 2)
    cols[:, B_N1] = bg(b_ih1, 2)
    cols[0:6, B_HEAD] = b_out
    cols[6, B_HEAD] = b_stop[0]
    cols[0:H, B_BASE0] = bg(b_ih0, 0)
    cols[0:H, B_BASE1] = bg(b_ih0, 1)
    cols[0:H, B_BASE2] = bg(b_ih0, 2)

    return {
        "whh0T": whh0T, "wfuseT": wfuseT, "wih1T": wih1T, "whh1T": whh1T,
        "wheadT": wheadT, "wlatT": wlatT, "ident": ident, "biases": cols,
    }


def kernel(latent, start_token, W_ih0, W_hh0, b_ih0, b_hh0,
           W_ih1, W_hh1, b_ih1, b_hh1, W_out, b_out, W_stop, b_stop,
           _return_raw=False, _trace=False):
    global _CACHED
    latent = np.asarray(latent, dtype=np.float32)
    B, T, D = latent.shape
    N = B * T
    assert N == NCORES * NS and D == LAT

    consts = _prep_weights(np.asarray(W_ih0), np.asarray(W_hh0),
                           np.asarray(b_ih0), np.asarray(b_hh0),
                           np.asarray(W_ih1), np.asarray(W_hh1),
                           np.asarray(b_ih1), np.asarray(b_hh1),
                           np.asarray(W_out), np.asarray(b_out),
                           np.asarray(W_stop), np.asarray(b_stop),
                           np.asarray(start_token))

    lat_flat = latent.reshape(N, D)
    in_maps = []
    for c in range(NCORES):
        lc = lat_flat[c * NS:(c + 1) * NS]                  # [8192, 256]
        x = lc.reshape(ROUNDS, NBLK, FD, 2, P)
        latT = np.ascontiguousarray(np.transpose(x, (0, 1, 3, 4, 2)))
        in_maps.append({"latT": latT, **consts})

    if _CACHED is None:
        _CACHED = _build_program()
    nc = _CACHED

    res = bass_utils.run_bass_kernel_spmd(
        nc, in_maps, core_ids=list(range(NCORES)), trace=_trace)

    full = np.concatenate([res.results[c]["outsd"] for c in range(NCORES)],
                          axis=2)                           # [K, 7, N]
    states = np.ascontiguousarray(
        np.transpose(full[:, 0:6, :], (2, 0, 1))).reshape(B, T, K, 6)
    logits = np.transpose(full[:, 6, :], (1, 0))            # [N, K]
    stops = (1.0 / (1.0 + np.exp(-logits.astype(np.float64)))).astype(
        np.float32).reshape(B, T, K)
    if _return_raw:
        return (states, stops), res
    return states, stops


# revision 13
# speedup vs baseline: 4.8850x; 4.8850x over previous
"""Trainium2 Bass kernel for a 2-layer GRU autoregressive decoder.

Problem shape: latent [512, 128, 256] -> fused batch N = 65536 independent
samples, K = 50 sequential decode steps each. Per step, per sample:
  x   = concat(lat, prev)                       # prev = previous state output
  h0  = GRUCell(x,  h0)   (hidden 128)
  h1  = GRUCell(h0, h1)   (hidden 128)
  out = W_out @ h1 + b_out   (6)  ;  stop = sigmoid(W_stop @ h1 + b_stop)

Device strategy (data parallel over 8 cores, 8192 samples each):
  * Layout: feature dim on SBUF partitions, samples on the free dim.
  * base = W_ih0[:, :256] @ lat^T + b_ih0 precomputed once per sample
    (latent is constant across the 50 steps).
  * prev only enters through W_ih0[:, 256:262] @ (W_out @ h1 + b_out), so
    W_fuse = W_ih0[:, 256:262] @ W_out turns the feedback into a plain
    128-contraction matmul on h1 (head taken off the critical path).
    Constant terms (b_fuse, and step-0's W_ih0p @ start_token) become ACT
    bias vectors.
  * Per 512-sample block and step: all gate pre-activations accumulate in
    PSUM (weight matmuls + identity-matmul adds of SBUF tensors), sigmoids
    and tanhs on ScalarE with fused per-partition biases,
    (hn + b_hh_n) * r fused into one scalar_tensor_tensor on VectorE, and
    the GRU state update h' = n + z*(h-n) done 4-blocks-wide on VectorE.
  * Head rows [out(6); stop_logit(1)] are evacuated PSUM->SBUF with the
    output bias fused, then DMAd to DRAM as [K, 7, 8192]; the final
    transpose to [B, T, K, ...] and the stop sigmoid are host-side
    reshaping of the gathered result.
"""

import sys

if "/opt/trn_rl_repo" not in sys.path:
    sys.path.insert(0, "/opt/trn_rl_repo")

import numpy as np

import concourse.bacc as bacc
import concourse.tile as tile
from concourse import mybir
from concourse import bass_utils

P = 128
FD = 512                 # samples per block (one PSUM bank of fp32)
GRP = 2                  # blocks per update group (wide DVE ops)
R = 4096                 # samples per round (SBUF residency of base/h)
ROUNDS = 2
NBLK = R // FD           # 8 blocks per round
K = 50                   # decode steps
NCORES = 8
NS = ROUNDS * R          # samples per core = 8192
LAT = 256
H = 128
F32 = mybir.dt.float32

AF = mybir.ActivationFunctionType
ALU = mybir.AluOpType

# bias column indices in the packed [128, 15] bias matrix
B_R0_K0, B_R0, B_Z0_K0, B_Z0, B_STT0, B_N0_K0, B_N0 = 0, 1, 2, 3, 4, 5, 6
B_R1, B_Z1, B_STT1, B_N1, B_HEAD = 7, 8, 9, 10, 11
B_BASE0, B_BASE1, B_BASE2 = 12, 13, 14
NBIAS = 15

_CACHED = None


def _build_program():
    nc = bacc.Bacc("TRN2", target_bir_lowering=False, debug=False,
                   num_devices=NCORES)

    latT_d = nc.dram_tensor("latT", [ROUNDS, NBLK, 2, P, FD], F32R,
                            kind="ExternalInput")
    whh0_d = nc.dram_tensor("whh0T", [P, 3, H], F32R, kind="ExternalInput")
    wfuse_d = nc.dram_tensor("wfuseT", [P, 3, H], F32R, kind="ExternalInput")
    wih1_d = nc.dram_tensor("wih1T", [P, 3, H], F32R, kind="ExternalInput")
    whh1_d = nc.dram_tensor("whh1T", [P, 3, H], F32R, kind="ExternalInput")
    whead_d = nc.dram_tensor("wheadT", [P, 7], F32R, kind="ExternalInput")
    wlat_d = nc.dram_tensor("wlatT", [P, 2, 3, H], F32R, kind="ExternalInput")
    ident_d = nc.dram_tensor("ident", [P, P], F32R, kind="ExternalInput")
    bias_d = nc.dram_tensor("biases", [P, NBIAS], F32, kind="ExternalInput")
    out_d = nc.dram_tensor("outsd", [K, 7, NS], F32, kind="ExternalOutput")
    out_ap = out_d.ap()

    W = GRP * FD  # group width for wide ACT/DVE ops

    with tile.TileContext(nc) as tc:
        with (
            tc.tile_pool(name="consts", bufs=1) as consts,
            tc.tile_pool(name="statep", bufs=1) as statep,
            tc.tile_pool(name="basep", bufs=1) as basep,
            tc.tile_pool(name="widep", bufs=2) as widep,
            tc.tile_pool(name="lats", bufs=2) as lats,
            tc.tile_pool(name="outsp", bufs=2) as outsp,
            tc.tile_pool(name="psum", bufs=4, space="PSUM") as psum,
        ):
            whh0 = consts.tile([P, 3, H], F32R)
            nc.sync.dma_start(whh0[:], whh0_d.ap())
            wfuse = consts.tile([P, 3, H], F32R)
            nc.sync.dma_start(wfuse[:], wfuse_d.ap())
            wih1 = consts.tile([P, 3, H], F32R)
            nc.sync.dma_start(wih1[:], wih1_d.ap())
            whh1 = consts.tile([P, 3, H], F32R)
            nc.sync.dma_start(whh1[:], whh1_d.ap())
            whead = consts.tile([P, 7], F32R)
            nc.sync.dma_start(whead[:], whead_d.ap())
            wlat = consts.tile([P, 2, 3, H], F32R)
            nc.sync.dma_start(wlat[:], wlat_d.ap())
            ident = consts.tile([P, P], F32R)
            nc.sync.dma_start(ident[:], ident_d.ap())
            biases = consts.tile([P, NBIAS], F32)
            nc.sync.dma_start(biases[:], bias_d.ap())

            def bcol(j, parts=P):
                return biases[0:parts, j:j + 1]

            def mm(out, lhsT, rhs, start, stop):
                if lhsT.dtype != F32R:
                    lhsT = _r(lhsT)
                if rhs.dtype != F32R:
                    rhs = _r(rhs)
                nc.tensor.matmul(out, lhsT, rhs, start=start, stop=stop)

            def mmgroup(out, pairs):
                for i, (l, rr) in enumerate(pairs):
                    mm(out, l, rr, start=(i == 0), stop=(i == len(pairs) - 1))

            for rnd in range(ROUNDS):
                # h tiles are first written at step 0 (h=0 terms dropped),
                # so no memset is needed.
                h0 = statep.tile([P, R], F32, tag="h0")
                h1 = statep.tile([P, R], F32, tag="h1")
                base = basep.tile([P, 3, R], F32, tag="base")

                # ---- precompute base = W_lat @ lat^T + b_ih0 ----
                for b in range(NBLK):
                    cs = slice(b * FD, (b + 1) * FD)
                    lat0 = lats.tile([P, FD], F32R, tag="lat0")
                    nc.sync.dma_start(lat0[:], latT_d.ap()[rnd, b, 0])
                    lat1 = lats.tile([P, FD], F32R, tag="lat1")
                    nc.sync.dma_start(lat1[:], latT_d.ap()[rnd, b, 1])
                    for g in range(3):
                        ps = psum.tile([P, FD], F32, tag="ps")
                        mmgroup(ps[:], [(wlat[:, 0, g, :], lat0[:]),
                                        (wlat[:, 1, g, :], lat1[:])])
                        nc.scalar.activation(_r(base[:, g, cs]), ps[:],
                                             AF.Identity,
                                             bias=bcol(B_BASE0 + g))

                # ---- decode steps: software-pipelined group tasks ----
                # L0(task t) || L1(task t-1) || head(task t-2) so the PE
                # in-order queue never sits behind an h-update chain.
                NG = NBLK // GRP

                def qslices(g):
                    out = []
                    for q in range(GRP):
                        b = g * GRP + q
                        out.append((slice(b * FD, (b + 1) * FD),      # in R
                                    slice(q * FD, (q + 1) * FD)))     # in W
                    return out

                state = {}

                def emit_L0(k, g):
                    gcs = slice(g * W, (g + 1) * W)
                    ps_r = psum.tile([P, W], F32, tag="ps")
                    ps_z = psum.tile([P, W], F32, tag="ps")
                    for cs, qs in qslices(g):
                        hp = [] if k == 0 else [(whh0[:, 0, :], h0[:, cs]),
                                                (wfuse[:, 0, :], h1[:, cs])]
                        mmgroup(ps_r[:, qs], hp + [(ident[:], base[:, 0, cs])])
                        hp = [] if k == 0 else [(whh0[:, 1, :], h0[:, cs]),
                                                (wfuse[:, 1, :], h1[:, cs])]
                        mmgroup(ps_z[:, qs], hp + [(ident[:], base[:, 1, cs])])
                    r0w = widep.tile([P, W], F32, tag="r0w")
                    nc.scalar.activation(r0w[:], ps_r[:], AF.Sigmoid,
                                         bias=bcol(B_R0_K0 if k == 0 else B_R0))
                    z0w = widep.tile([P, W], F32, tag="z0w")
                    nc.scalar.activation(z0w[:], ps_z[:], AF.Sigmoid,
                                         bias=bcol(B_Z0_K0 if k == 0 else B_Z0))

                    tmp0w = widep.tile([P, W], F32, tag="tmp0w")
                    if k == 0:
                        nc.vector.tensor_scalar_mul(_r(tmp0w[:]), r0w[:],
                                                    bcol(B_STT0))
                    else:
                        ps_hn = psum.tile([P, W], F32, tag="ps")
                        for cs, qs in qslices(g):
                            mm(ps_hn[:, qs], whh0[:, 2, :], h0[:, cs],
                               start=True, stop=True)
                        nc.vector.scalar_tensor_tensor(
                            _r(tmp0w[:]), ps_hn[:], bcol(B_STT0), r0w[:],
                            op0=ALU.add, op1=ALU.mult)

                    ps_in = psum.tile([P, W], F32, tag="ps")
                    for cs, qs in qslices(g):
                        hp = [] if k == 0 else [(wfuse[:, 2, :], h1[:, cs])]
                        mmgroup(ps_in[:, qs],
                                hp + [(ident[:], base[:, 2, cs]),
                                      (ident[:], tmp0w[:, qs])])
                    n0w = widep.tile([P, W], F32, tag="n0w")
                    nc.scalar.activation(n0w[:], ps_in[:], AF.Tanh,
                                         bias=bcol(B_N0_K0 if k == 0 else B_N0))

                    # h0' = n + z*(h0 - n)
                    d0 = widep.tile([P, W], F32, tag="d0", bufs=1)
                    if k == 0:
                        nc.vector.tensor_scalar_mul(d0[:], n0w[:], -1.0)
                    else:
                        nc.vector.tensor_sub(d0[:], h0[:, gcs], n0w[:])
                    nc.vector.tensor_mul(d0[:], z0w[:], d0[:])
                    nc.vector.tensor_add(_r(h0[:, gcs]), n0w[:], d0[:])

                def emit_L1(k, g):
                    gcs = slice(g * W, (g + 1) * W)
                    ps_r = psum.tile([P, W], F32, tag="ps")
                    ps_z = psum.tile([P, W], F32, tag="ps")
                    for cs, qs in qslices(g):
                        hp = [] if k == 0 else [(whh1[:, 0, :], h1[:, cs])]
                        mmgroup(ps_r[:, qs], [(wih1[:, 0, :], h0[:, cs])] + hp)
                        hp = [] if k == 0 else [(whh1[:, 1, :], h1[:, cs])]
                        mmgroup(ps_z[:, qs], [(wih1[:, 1, :], h0[:, cs])] + hp)
                    r1w = widep.tile([P, W], F32, tag="r1w")
                    nc.scalar.activation(r1w[:], ps_r[:], AF.Sigmoid,
                                         bias=bcol(B_R1))
                    z1w = widep.tile([P, W], F32, tag="z1w")
                    nc.scalar.activation(z1w[:], ps_z[:], AF.Sigmoid,
                                         bias=bcol(B_Z1))

                    tmp1w = widep.tile([P, W], F32, tag="tmp1w")
                    if k == 0:
                        nc.vector.tensor_scalar_mul(_r(tmp1w[:]), r1w[:],
                                                    bcol(B_STT1))
                    else:
                        ps_hn = psum.tile([P, W], F32, tag="ps")
                        for cs, qs in qslices(g):
                            mm(ps_hn[:, qs], whh1[:, 2, :], h1[:, cs],
                               start=True, stop=True)
                        nc.vector.scalar_tensor_tensor(
                            _r(tmp1w[:]), ps_hn[:], bcol(B_STT1), r1w[:],
                            op0=ALU.add, op1=ALU.mult)

                    ps_in = psum.tile([P, W], F32, tag="ps")
                    for cs, qs in qslices(g):
                        mmgroup(ps_in[:, qs], [(wih1[:, 2, :], h0[:, cs]),
                                               (ident[:], tmp1w[:, qs])])
                    n1w = widep.tile([P, W], F32, tag="n1w")
                    nc.scalar.activation(n1w[:], ps_in[:], AF.Tanh,
                                         bias=bcol(B_N1))

                    d1 = widep.tile([P, W], F32, tag="d1", bufs=1)
                    if k == 0:
                        nc.vector.tensor_scalar_mul(d1[:], n1w[:], -1.0)
                    else:
                        nc.vector.tensor_sub(d1[:], h1[:, gcs], n1w[:])
                    nc.vector.tensor_mul(d1[:], z1w[:], d1[:])
                    nc.vector.tensor_add(_r(h1[:, gcs]), n1w[:], d1[:])

                def emit_head(k, g):
                    ps_h = psum.tile([7, W], F32, tag="ps")
                    for cs, qs in qslices(g):
                        mm(ps_h[:, qs], whead[:], h1[:, cs],
                           start=True, stop=True)
                    ost = outsp.tile([7, W], F32, tag="ost")
                    nc.scalar.activation(ost[:], ps_h[:], AF.Identity,
                                         bias=bcol(B_HEAD, parts=7))
                    col0 = rnd * R + g * W
                    nc.sync.dma_start(out_ap[k, :, col0:col0 + W], ost[:])

                tasks = [(k, g) for k in range(K) for g in range(NG)]
                for t in range(len(tasks) + 3):
                    if t < len(tasks):
                        emit_L0(*tasks[t])
                    if 0 <= t - 1 < len(tasks):
                        emit_L1(*tasks[t - 1])
                    if 0 <= t - 3 < len(tasks):
                        emit_head(*tasks[t - 3])

    nc.compile()
    return nc


def _prep_weights(W_ih0, W_hh0, b_ih0, b_hh0, W_ih1, W_hh1, b_ih1, b_hh1,
                  W_out, b_out, W_stop, b_stop, start_token):
    f32 = np.float32
    Wlat = np.ascontiguousarray(W_ih0[:, :LAT])
    Wp = W_ih0[:, LAT:].astype(np.float64)
    W_fuse = (Wp @ W_out.astype(np.float64)).astype(f32)
    b_fuse = (Wp @ b_out.astype(np.float64)).astype(f32)
    c0 = (Wp @ start_token[0].astype(np.float64)).astype(f32)

    def lhsT3(W):  # [384, 128] -> [128(contract), 3(gate), 128(out)]
        return np.ascontiguousarray(
            np.transpose(W.reshape(3, H, H), (2, 0, 1)).astype(f32))

    whh0T = lhsT3(W_hh0)
    wfuseT = lhsT3(W_fuse)
    wih1T = lhsT3(W_ih1)
    whh1T = lhsT3(W_hh1)
    wheadT = np.ascontiguousarray(
        np.vstack([W_out, W_stop]).T.astype(f32))          # [128, 7]
    # [128(contract within chunk), 2(chunk), 3(gate), 128(out)]
    wlatT = np.ascontiguousarray(
        np.transpose(Wlat.reshape(3, H, 2, P), (3, 2, 0, 1)).astype(f32))
    ident = np.eye(P, dtype=f32)

    def bg(v, g):
        return v[g * H:(g + 1) * H]

    cols = np.zeros((P, NBIAS), dtype=f32)
    cols[:, B_R0_K0] = bg(b_hh0, 0) + bg(c0, 0)
    cols[:, B_R0] = bg(b_hh0, 0) + bg(b_fuse, 0)
    cols[:, B_Z0_K0] = bg(b_hh0, 1) + bg(c0, 1)
    cols[:, B_Z0] = bg(b_hh0, 1) + bg(b_fuse, 1)
    cols[:, B_STT0] = bg(b_hh0, 2)
    cols[:, B_N0_K0] = bg(c0, 2)
    cols[:, B_N0] = bg(b_fuse, 2)
    cols[:, B_R1] = bg(b_ih1, 0) + bg(b_hh1, 0)
    cols[:, B_Z1] = bg(b_ih1, 1) + bg(b_hh1, 1)
    cols[:, B_STT1] = bg(b_hh1, 2)
    cols[:, B_N1] = bg(b_ih1, 2)
    cols[0:6, B_HEAD] = b_out
    cols[6, B_HEAD] = b_stop[0]
    cols[0:H, B_BASE0] = bg(b_ih0, 0)
    cols[0:H, B_BASE1] = bg(b_ih0, 1)
    cols[0:H, B_BASE2] = bg(b_ih0, 2)

    return {
        "whh0T": whh0T, "wfuseT": wfuseT, "wih1T": wih1T, "whh1T": whh1T,
        "wheadT": wheadT, "wlatT": wlatT, "ident": ident, "biases": cols,
    }


def kernel(latent, start_token, W_ih0, W_hh0, b_ih0, b_hh0,
           W_ih1, W_hh1, b_ih1, b_hh1, W_out, b_out, W_stop, b_stop,
           _return_raw=False, _trace=False):
    global _CACHED
    latent = np.asarray(latent, dtype=np.float32)
    B, T, D = latent.shape
    N = B * T
    assert N == NCORES * NS and D == LAT

    consts = _prep_weights(np.asarray(W_ih0), np.asarray(W_hh0),
                           np.asarray(b_ih0), np.asarray(b_hh0),
                           np.asarray(W_ih1), np.asarray(W_hh1),
                           np.asarray(b_ih1), np.asarray(b_hh1),
                           np.asarray(W_out), np.asarray(b_out),
                           np.asarray(W_stop), np.asarray(b_stop),
                           np.asarray(start_token))

    lat_flat = latent.reshape(N, D)
    in_maps = []
    for c in range(NCORES):
        lc = lat_flat[c * NS:(c + 1) * NS]                  # [8192, 256]
        x = lc.reshape(ROUNDS, NBLK, FD, 2, P)
        latT = np.ascontiguousarray(np.transpose(x, (0, 1, 3, 4, 2)))
        in_maps.append({"latT": latT, **consts})

    if _CACHED is None:
        _CACHED = _build_program()
    nc = _CACHED

    res = bass_utils.run_bass_kernel_spmd(
        nc, in_maps, core_ids=list(range(NCORES)), trace=_trace)

    full = np.concatenate([res.results[c]["outsd"] for c in range(NCORES)],
                          axis=2)                           # [K, 7, N]
    states = np.ascontiguousarray(
        np.transpose(full[:, 0:6, :], (2, 0, 1))).reshape(B, T, K, 6)
    logits = np.transpose(full[:, 6, :], (1, 0))            # [N, K]
    stops = (1.0 / (1.0 + np.exp(-logits.astype(np.float64)))).astype(
        np.float32).reshape(B, T, K)
    if _return_raw:
        return (states, stops), res
    return states, stops
